# revision 22
# baseline (speedup 1.0000x reference)
"""GQA attention (Llama-style) on 8 Trainium2 NeuronCores.

Tensor-parallel over heads: core c owns q-heads [3c, 3c+1, 3c+2] and KV
head c. Each core computes a partial output contribution via its slice of
Wo (row-parallel); the host sums the 8 partials.

All matmul operands are bf16 (rel err ~6e-3 vs the fp32 reference, well
under the 2e-2 gate). Inputs are pre-laid-out on the host partition-major
so every DMA descriptor is >=1KB contiguous. Transposes (V and the
attention output) run on the DMA engines' XBAR path instead of the PE.

Schedule notes (v2):
- Projections are chunked 512 seq-cols at a time (8 chunks) so the first
  chain only needs 3MB of x + wk before it can finish; DMA emission is
  ordered x-first/wk-first across the three queues so the PE starts real
  work at ~3us instead of ~39us (and HAM stays warm).
- x tile pool holds 2 full chunks so prefetch never blocks on recycle.
- wo is preloaded during scope 1 so the b0 O-projection can fill the
  scope seam while b1's first score exps stream.
- Attention (h, half) units are emitted halves-first so O-proj chunks
  0-7 unblock after the first three pv_blocks.
- The last two output chunks split their store DMA in half across two
  queues to shorten the drain tail.

Shapes (hardcoded per the problem spec):
  hidden_states [2, 2048, 3072] f32, attention_mask [2,1,2048,2048] (zeros),
  Wq [3072, 3072], Wk/Wv [3072, 1024], Wo [3072, 3072] -> out [2, 2048, 3072].
"""

import ml_dtypes
import numpy as np

B, S, H = 2, 2048, 3072
NH, NKV, HD = 24, 8, 128
HPC = NH // 8        # q-heads per core
NT = H // 128        # 24 h-tiles of the hidden dim
NKT = S // 128       # 16 k-tiles of the sequence
NCH = 4              # projection chunks per batch
CW = S // NCH        # 512 seq-cols per chunk
NG = 6               # x DMA groups per chunk (512KB each)
GT = NT // NG        # 4 h-tiles per group
SCALE = float(1.0 / np.sqrt(HD))

_CACHE = {}


def _build():
    import concourse.mybir as mybir
    import concourse.tile as tile
    from concourse import bacc

    f32 = mybir.dt.float32
    bf16 = mybir.dt.bfloat16
    Exp = mybir.ActivationFunctionType.Exp

    nc = bacc.Bacc(None, target_bir_lowering=False)

    # Host pre-transposed, partition-major layouts (see _prep_inputs()).
    xt_d = nc.dram_tensor("xt", [B, 128, NT, S], bf16, kind="ExternalInput")
    wq_d = nc.dram_tensor("wq", [128, NT, HPC * HD], bf16, kind="ExternalInput")
    wk_d = nc.dram_tensor("wk", [128, NT, HD], bf16, kind="ExternalInput")
    wv_d = nc.dram_tensor("wv", [128, NT, HD], bf16, kind="ExternalInput")
    wo_d = nc.dram_tensor("wo", [128, HPC, H], bf16, kind="ExternalInput")
    out_d = nc.dram_tensor("out", [B, S, H], bf16, kind="ExternalOutput")

    with tile.TileContext(nc) as tc:
        with (
            tc.tile_pool(name="qkv", bufs=1) as qkvp,
            tc.tile_pool(name="ut0", bufs=1) as utp0,
            tc.tile_pool(name="wop", bufs=1) as wop,
            tc.tile_pool(name="small", bufs=4) as smallp,
            tc.tile_pool(name="psS", bufs=2, space="PSUM") as psS,
            tc.tile_pool(name="psU", bufs=2, space="PSUM") as psU,
        ):
            # Persistent per-(b,head) projections; partition dim is head_dim.
            qt = [qkvp.tile([128, S], bf16, name=f"qt{i}", tag="qt", bufs=B * HPC)
                  for i in range(B * HPC)]
            kt = [qkvp.tile([128, S], bf16, name=f"kt{i}", tag="kt", bufs=B)
                  for i in range(B)]
            # V with a fused ones column: [s-tile partition, k-tile, 129]
            vaug = [qkvp.tile([128, NKT, HD + 1], bf16, name=f"va{i}", tag="va",
                              bufs=B)
                    for i in range(B)]
            ut0 = [utp0.tile([128, S], bf16, name=f"u0{h}", tag="u0", bufs=HPC)
                   for h in range(HPC)]
            # wo lives in the outer scope so it can be preloaded in scope 1
            # and used by the O-projection in scope 2.
            wo_sb = wop.tile([128, HPC, H], bf16, name="wo", tag="wo")

            # ---------- emission helpers ----------
            def scores_block(pkp, pk_out, b, h, half):
                """Q@K^T for 1024 queries; exp on ACT -> pk strips (bf16)."""
                q0 = half * 1024
                qi = b * HPC + h
                for k in range(NKT):
                    stp = psS.tile([128, 1024], f32, name="stp", tag="st")
                    ksl = kt[b][:, k * 128:(k + 1) * 128]
                    nc.tensor.matmul(stp[:, 0:512], ksl,
                                     qt[qi][:, q0:q0 + 512],
                                     start=True, stop=True)
                    nc.tensor.matmul(stp[:, 512:1024], ksl,
                                     qt[qi][:, q0 + 512:q0 + 1024],
                                     start=True, stop=True)
                    pk = pkp.tile([128, 1024], bf16, name="pk", tag="pk")
                    nc.scalar.activation(pk[:], stp[:], Exp, scale=SCALE)
                    pk_out[k] = pk

            def pv_block(pks, ut, b, half, tsplit=False):
                """P@V_aug for 1024 queries; normalize; XBAR-transpose to ut.

                tsplit: alternate the transposes across sync+scalar (only
                safe once the scalar engine's exp stream has drained).
                """
                q0 = half * 1024
                for qtl in range(8):
                    up = psU.tile([128, HD + 1], f32, name="up", tag="u")
                    for k in range(NKT):
                        nc.tensor.matmul(up[:],
                                         pks[k][:, qtl * 128:(qtl + 1) * 128],
                                         vaug[b][:, k, :],
                                         start=(k == 0), stop=(k == NKT - 1))
                    rs = smallp.tile([128, 1], f32, name="rs", tag="rs")
                    nc.vector.reciprocal(rs[:], up[:, HD:HD + 1])
                    un = smallp.tile([128, 128], bf16, name="un", tag="un",
                                     bufs=6)
                    nc.vector.tensor_scalar_mul(un[:], up[:, 0:HD], rs[:])
                    teng = nc.scalar if (tsplit and qtl % 2) else nc.sync
                    teng.dma_start_transpose(
                        ut[:, q0 + qtl * 128:q0 + (qtl + 1) * 128], un[:])

            # ---------- scope 1: projections woven with b0 attention ----------
            pk0 = {}   # (h, half) -> list of pk strips for b=0
            with (
                tc.tile_pool(name="wts", bufs=1) as wp,
                tc.tile_pool(name="xts", bufs=2 * NG) as xtp,
                tc.tile_pool(name="vt", bufs=1) as vtp,
                tc.tile_pool(name="pk0", bufs=18) as pk0p,
                tc.tile_pool(name="psA", bufs=2, space="PSUM") as psA,
            ):
                wq_sb = wp.tile([128, NT, HPC * HD], bf16, name="wq", tag="wq")
                wk_sb = wp.tile([128, NT, HD], bf16, name="wk", tag="wk")
                wv_sb = wp.tile([128, NT, HD], bf16, name="wv", tag="wv")
                vt = vtp.tile([128, S], bf16, name="vt", tag="vt", bufs=1)

                # PE warmup first: dummy matmuls keep the PE busy (and HAM
                # warming) from t~0 while the first DMAs land. Output
                # overwritten by the real O-projection later.
                wu = wp.tile([128, 512], bf16, name="wu", tag="wu")
                nc.vector.memset(wu[:], 0.0)
                pwu = psA.tile([128, 512], f32, name="pwu", tag="pp")
                for i in range(40):
                    nc.tensor.matmul(pwu[:, 0:128], wu[:, 0:128], wu[:, 0:128],
                                     start=(i == 0), stop=(i == 39))
                nc.vector.tensor_copy(wu[:, 0:128], pwu[:, 0:128])

                def gate(dst):
                    """Tiny copy into dst's first column, reading the warmup
                    output. Forces the subsequent DMA into dst (WAW) to wait
                    until the warmup drains, so the scheduler cannot hoist
                    bulk weight loads into the critical x-fill window."""
                    nc.vector.tensor_copy(dst, wu[:, 1:2])

                # ones columns for the fused softmax denominator
                for b in range(B):
                    nc.vector.memset(vaug[b][:, :, HD:HD + 1], 1.0)

                def load_chunk(b, c, head=False, gated=False):
                    """6 group-DMAs of [128, 4 h-tiles, 512 cols] (512KB)."""
                    sl = slice(c * CW, (c + 1) * CW)
                    xts = []
                    for g in range(NG):
                        xtile = xtp.tile([128, GT, CW], bf16, name=f"x{g}",
                                         tag="x")
                        if head:
                            eng = [nc.scalar, nc.gpsimd, nc.sync][g % 3]
                        else:
                            eng = nc.gpsimd
                        if gated:
                            gate(xtile[:, 0, 0:1])
                        eng.dma_start(xtile[:],
                                      xt_d[b, :, g * GT:(g + 1) * GT, sl])
                        xts.append(xtile)
                    return xts

                GRPS = [HPC, HPC + 1, 0, 1, 2]  # K, V first, then q-heads

                def chain(b, c, xts, grp):
                    """One 24-matmul accumulation chain -> qt/kt/vt slice."""
                    pp = psA.tile([128, CW], f32, name="pp", tag="pp")
                    for t in range(NT):
                        if grp < HPC:
                            w_sl = wq_sb[:, t, grp * HD:(grp + 1) * HD]
                        elif grp == HPC:
                            w_sl = wk_sb[:, t, :]
                        else:
                            w_sl = wv_sb[:, t, :]
                        nc.tensor.matmul(pp[:], w_sl,
                                         xts[t // GT][:, t % GT, :],
                                         start=(t == 0), stop=(t == NT - 1))
                    osl = slice(c * CW, (c + 1) * CW)
                    if grp < HPC:
                        nc.vector.tensor_copy(qt[b * HPC + grp][:, osl], pp[:])
                    elif grp == HPC:
                        nc.vector.tensor_copy(kt[b][:, osl], pp[:])
                    else:
                        nc.vector.tensor_copy(vt[:, osl], pp[:])

                def v_fixup(b, c):
                    # XBAR transpose needs a 256B-aligned destination; stage
                    # at offset 0 and let gpsimd scatter into vaug.
                    for st in range(4 * c, 4 * c + 4):
                        tst = smallp.tile([128, 128], bf16, name="tst",
                                          tag="tst", bufs=4)
                        nc.sync.dma_start_transpose(
                            tst[:], vt[:, st * 128:(st + 1) * 128])
                        nc.gpsimd.tensor_copy(vaug[b][:, st, 0:HD], tst[:])

                # ---- DMA emission: wk (split across two rings) + x chunk 0
                # at high priority; every other load is gated behind the
                # warmup so the scheduler cannot hoist it into the critical
                # x-fill window (rings serialize ops, so early bulk weight
                # issues delay every chunk-0 completion).
                with tc.high_priority():
                    nc.scalar.dma_start(wk_sb[:, 0:12, :], wk_d[:, 0:12, :])
                    nc.sync.dma_start(wk_sb[:, 12:24, :], wk_d[:, 12:24, :])
                    xts00 = load_chunk(0, 0, head=True)
                gate(wv_sb[:, 0, 0:1])
                nc.gpsimd.dma_start(wv_sb[:], wv_d[:])
                for wc in range(4):
                    eng = [nc.sync, nc.scalar, nc.gpsimd, nc.sync][wc]
                    gate(wq_sb[:, wc * 6, 0:1])
                    eng.dma_start(wq_sb[:, wc * 6:(wc + 1) * 6, :],
                                  wq_d[:, wc * 6:(wc + 1) * 6, :])

                # b0 projection chunks
                for grp in GRPS:
                    chain(0, 0, xts00, grp)
                    if grp == HPC + 1:
                        v_fixup(0, 0)
                for c in range(1, NCH):
                    xts = load_chunk(0, c, head=True, gated=(c == 1))
                    for grp in GRPS:
                        chain(0, c, xts, grp)
                        if grp == HPC + 1:
                            v_fixup(0, c)

                # b1 chains woven with b0 attention
                rest = []
                xts10 = load_chunk(1, 0)
                rest += [(1, 0, xts10, grp) for grp in GRPS]
                ri = [0]
                loaded = [1]  # b1 chunks loaded so far

                def emit_chains(n):
                    for _ in range(n):
                        if ri[0] < len(rest):
                            b, c, xts, grp = rest[ri[0]]
                            chain(b, c, xts, grp)
                            if grp == HPC + 1:
                                v_fixup(1, c)
                            ri[0] += 1
                        # 2 chains into chunk c -> prefetch chunk c+1
                        if (loaded[0] < NCH
                                and ri[0] >= 5 * (loaded[0] - 1) + 2):
                            xts_n = load_chunk(1, loaded[0])
                            rest.extend((1, loaded[0], xts_n, grp)
                                        for grp in GRPS)
                            loaded[0] += 1

                # weave: b0 attention between b1 projection chains;
                # halves-first so O-proj chunks unblock in order.
                seq0 = [(h, hf) for hf in range(2) for h in range(HPC)]
                budget = [3, 3, 3, 3, 4, 4]
                for i, (h, half) in enumerate(seq0):
                    pk0[(h, half)] = [None] * NKT
                    scores_block(pk0p, pk0[(h, half)], 0, h, half)
                    emit_chains(budget[i])
                    pv_block(pk0[(h, half)], ut0[h], 0, half)
                    if i == 0:
                        # preload wo for scope 2's O-projection; gated on
                        # b1's first K-chain output so its 2.4MB cannot be
                        # hoisted into the startup's x fill
                        nc.vector.tensor_copy(wo_sb[:, 0, 0:1],
                                              kt[1][:, 0:1])
                        nc.gpsimd.dma_start(wo_sb[:], wo_d[:])
                assert ri[0] == 20 and len(rest) == 20 and loaded[0] == NCH

            # ---------- scope 2: b0 O-proj woven with b1 attention ----------
            with (
                tc.tile_pool(name="ut1", bufs=1) as utp1,
                tc.tile_pool(name="pk1", bufs=50) as pk1p,
                tc.tile_pool(name="ost", bufs=3) as ostp,
                tc.tile_pool(name="psO", bufs=2, space="PSUM") as psO,
            ):
                ut1 = [utp1.tile([128, S], bf16, name=f"u1{h}", tag="u1",
                                 bufs=HPC)
                       for h in range(HPC)]

                def oproj_chunk(ut, b, sc, late=False, split_tail=False):
                    """One 128-query row block x full H output.

                    One output DMA per row block (6KB/partition descriptors).
                    During b=0's O-proj the scalar engine is saturated with
                    b=1's exp stream, so those PSUM copies stay on vector.
                    Once the exps drain (`late` b=0 chunks, all b=1 chunks)
                    most copies move to scalar so the vector queue stays
                    clear for the pv normalize chains, and the by-then-idle
                    psS score banks give a 4-deep PSUM rotation.
                    split_tail: issue the store in two halves on separate
                    queues to shorten the end-of-kernel drain.
                    """
                    ssl = slice(sc * 128, (sc + 1) * 128)
                    ob = ostp.tile([128, H], bf16, name="ob", tag="ob")
                    for q in range(2 * HPC):
                        if (b == 1 or late) and q % 2 == 1:
                            op = psS.tile([128, 1024], f32, name="stp",
                                          tag="st")[:, 0:512]
                        else:
                            op = psO.tile([128, 512], f32, name="op", tag="o")
                        n0 = q * 512
                        for dq in range(HPC):
                            nc.tensor.matmul(op[:], ut[dq][:, ssl],
                                             wo_sb[:, dq, n0:n0 + 512],
                                             start=(dq == 0),
                                             stop=(dq == HPC - 1))
                        on_scalar = (q % 3 != 0) if late else (
                            b == 1 and q % 2 == 1)
                        if on_scalar:
                            nc.scalar.copy(ob[:, n0:n0 + 512], op[:])
                        else:
                            nc.vector.tensor_copy(ob[:, n0:n0 + 512], op[:])
                        if split_tail and q == 2:
                            nc.gpsimd.dma_start(out_d[b, ssl, 0:1536],
                                                ob[:, 0:1536])
                    if split_tail:
                        nc.sync.dma_start(out_d[b, ssl, 1536:H],
                                          ob[:, 1536:H])
                    else:
                        deng = nc.gpsimd if (b == 0 or sc % 2 == 0) else nc.sync
                        deng.dma_start(out_d[b, ssl, :], ob[:])

                sc0 = [0]

                def oproj0(n, late=False):
                    for _ in range(n):
                        if sc0[0] < S // 128:
                            oproj_chunk(ut0, 0, sc0[0], late=late)
                            sc0[0] += 1

                seq1 = [(h, hf) for hf in range(2) for h in range(HPC)]
                pk1 = {}

                def scores1(i):
                    pk1[seq1[i]] = [None] * NKT
                    scores_block(pk1p, pk1[seq1[i]], 1, *seq1[i])

                scores1(0)
                oproj0(2)
                scores1(1)
                oproj0(2)
                for i in range(2, 6):
                    scores1(i)
                    oproj0(2)
                    ph, phalf = seq1[i - 2]
                    pv_block(pk1[(ph, phalf)], ut1[ph], 1, phalf)
                oproj0(2, late=True)
                pv_block(pk1[seq1[4]], ut1[seq1[4][0]], 1, seq1[4][1],
                         tsplit=True)
                oproj0(2, late=True)
                pv_block(pk1[seq1[5]], ut1[seq1[5][0]], 1, seq1[5][1],
                         tsplit=True)
                oproj0(S // 128, late=True)  # remainder of b0, if any
                for sc in range(S // 128):
                    oproj_chunk(ut1, 1, sc, split_tail=(sc >= 12))

    nc.compile()
    return nc


def kernel(hidden_states, attention_mask, Wq, Wk, Wv, Wo):
    import os
    import tempfile

    from concourse.bass_utils import run_bass_kernel_spmd

    # the neuron compile hook drops a scratch file into cwd
    if not os.access(os.getcwd(), os.W_OK):
        os.chdir(tempfile.mkdtemp())

    if "nc" not in _CACHE:
        _CACHE["nc"] = _build()
    nc = _CACHE["nc"]

    in_maps = _prep_inputs(hidden_states, Wq, Wk, Wv, Wo)
    res = run_bass_kernel_spmd(nc, in_maps, core_ids=list(range(8)))
    out = np.zeros((B, S, H), dtype=np.float32)
    for r in res.results:
        out += r["out"].astype(np.float32)
    return out


def _prep_inputs(hidden_states, Wq, Wk, Wv, Wo):
    bf = ml_dtypes.bfloat16
    hs = np.asarray(hidden_states, dtype=np.float32)
    # xt[b, p, t, s] = hs[b, s, t*128 + p]
    xt = np.ascontiguousarray(
        hs.transpose(0, 2, 1).reshape(B, NT, 128, S).transpose(0, 2, 1, 3)
    ).astype(bf)
    Wq = np.asarray(Wq, dtype=np.float32)
    Wk = np.asarray(Wk, dtype=np.float32)
    Wv = np.asarray(Wv, dtype=np.float32)
    Wo = np.asarray(Wo, dtype=np.float32)

    def wslice(W, c, width):
        # [H, width] -> [128, NT, width] partition-major
        ws = W[:, c * width:(c + 1) * width]
        return np.ascontiguousarray(
            ws.reshape(NT, 128, width).transpose(1, 0, 2)).astype(bf)

    in_maps = []
    for c in range(8):
        wo = Wo[c * HPC * HD:(c + 1) * HPC * HD, :]  # [384, H]
        wo = np.ascontiguousarray(
            wo.reshape(HPC, 128, H).transpose(1, 0, 2)).astype(bf)
        in_maps.append({
            "xt": xt,
            "wq": wslice(Wq, c, HPC * HD),
            "wk": wslice(Wk, c, HD),
            "wv": wslice(Wv, c, HD),
            "wo": wo,
        })
    return in_maps


# revision 25
# speedup vs baseline: 1.0009x; 1.0009x over previous
"""GQA attention (Llama-style) on 8 Trainium2 NeuronCores.

Tensor-parallel over heads: core c owns q-heads [3c, 3c+1, 3c+2] and KV
head c. Each core computes a partial output contribution via its slice of
Wo (row-parallel); the host sums the 8 partials.

All matmul operands are bf16 (rel err ~6e-3 vs the fp32 reference, well
under the 2e-2 gate). Inputs are pre-laid-out on the host partition-major
so every DMA descriptor is >=1KB contiguous. Transposes (V and the
attention output) run on the DMA engines' XBAR path instead of the PE.

Schedule notes (v2):
- Projections are chunked 512 seq-cols at a time (8 chunks) so the first
  chain only needs 3MB of x + wk before it can finish; DMA emission is
  ordered x-first/wk-first across the three queues so the PE starts real
  work at ~3us instead of ~39us (and HAM stays warm).
- x tile pool holds 2 full chunks so prefetch never blocks on recycle.
- wo is preloaded during scope 1 so the b0 O-projection can fill the
  scope seam while b1's first score exps stream.
- Attention (h, half) units are emitted halves-first so O-proj chunks
  0-7 unblock after the first three pv_blocks.
- The last two output chunks split their store DMA in half across two
  queues to shorten the drain tail.

Shapes (hardcoded per the problem spec):
  hidden_states [2, 2048, 3072] f32, attention_mask [2,1,2048,2048] (zeros),
  Wq [3072, 3072], Wk/Wv [3072, 1024], Wo [3072, 3072] -> out [2, 2048, 3072].
"""

import ml_dtypes
import numpy as np

B, S, H = 2, 2048, 3072
NH, NKV, HD = 24, 8, 128
HPC = NH // 8        # q-heads per core
NT = H // 128        # 24 h-tiles of the hidden dim
NKT = S // 128       # 16 k-tiles of the sequence
NCH = 4              # projection chunks per batch
CW = S // NCH        # 512 seq-cols per chunk
NG = 6               # x DMA groups per chunk (512KB each)
GT = NT // NG        # 4 h-tiles per group
SCALE = float(1.0 / np.sqrt(HD))

_CACHE = {}


def _build():
    import concourse.mybir as mybir
    import concourse.tile as tile
    from concourse import bacc

    f32 = mybir.dt.float32
    bf16 = mybir.dt.bfloat16
    Exp = mybir.ActivationFunctionType.Exp

    nc = bacc.Bacc(None, target_bir_lowering=False)

    # Host pre-transposed, partition-major layouts (see _prep_inputs()).
    xt_d = nc.dram_tensor("xt", [B, 128, NT, S], bf16, kind="ExternalInput")
    wq_d = nc.dram_tensor("wq", [128, NT, HPC * HD], bf16, kind="ExternalInput")
    wk_d = nc.dram_tensor("wk", [128, NT, HD], bf16, kind="ExternalInput")
    wv_d = nc.dram_tensor("wv", [128, NT, HD], bf16, kind="ExternalInput")
    wo_d = nc.dram_tensor("wo", [128, HPC, H], bf16, kind="ExternalInput")
    out_d = nc.dram_tensor("out", [B, S, H], bf16, kind="ExternalOutput")

    with tile.TileContext(nc) as tc:
        with (
            tc.tile_pool(name="qkv", bufs=1) as qkvp,
            tc.tile_pool(name="ut0", bufs=1) as utp0,
            tc.tile_pool(name="wop", bufs=1) as wop,
            tc.tile_pool(name="small", bufs=4) as smallp,
            tc.tile_pool(name="psS", bufs=2, space="PSUM") as psS,
            tc.tile_pool(name="psU", bufs=2, space="PSUM") as psU,
        ):
            # Persistent per-(b,head) projections; partition dim is head_dim.
            qt = [qkvp.tile([128, S], bf16, name=f"qt{i}", tag="qt", bufs=B * HPC)
                  for i in range(B * HPC)]
            kt = [qkvp.tile([128, S], bf16, name=f"kt{i}", tag="kt", bufs=B)
                  for i in range(B)]
            # V with a fused ones column: [s-tile partition, k-tile, 129]
            vaug = [qkvp.tile([128, NKT, HD + 1], bf16, name=f"va{i}", tag="va",
                              bufs=B)
                    for i in range(B)]
            ut0 = [utp0.tile([128, S], bf16, name=f"u0{h}", tag="u0", bufs=HPC)
                   for h in range(HPC)]
            # wo lives in the outer scope so it can be preloaded in scope 1
            # and used by the O-projection in scope 2.
            wo_sb = wop.tile([128, HPC, H], bf16, name="wo", tag="wo")

            # ---------- emission helpers ----------
            def scores_block(pkp, pk_out, b, h, half):
                """Q@K^T for 1024 queries; exp on ACT -> pk strips (bf16)."""
                q0 = half * 1024
                qi = b * HPC + h
                for k in range(NKT):
                    stp = psS.tile([128, 1024], f32, name="stp", tag="st")
                    ksl = kt[b][:, k * 128:(k + 1) * 128]
                    nc.tensor.matmul(stp[:, 0:512], ksl,
                                     qt[qi][:, q0:q0 + 512],
                                     start=True, stop=True)
                    nc.tensor.matmul(stp[:, 512:1024], ksl,
                                     qt[qi][:, q0 + 512:q0 + 1024],
                                     start=True, stop=True)
                    pk = pkp.tile([128, 1024], bf16, name="pk", tag="pk")
                    nc.scalar.activation(pk[:], stp[:], Exp, scale=SCALE)
                    pk_out[k] = pk

            def pv_block(pks, ut, b, half, tsplit=False):
                """P@V_aug for 1024 queries; normalize; XBAR-transpose to ut.

                tsplit: alternate the transposes across sync+scalar (only
                safe once the scalar engine's exp stream has drained).
                """
                q0 = half * 1024
                for qtl in range(8):
                    up = psU.tile([128, HD + 1], f32, name="up", tag="u")
                    for k in range(NKT):
                        nc.tensor.matmul(up[:],
                                         pks[k][:, qtl * 128:(qtl + 1) * 128],
                                         vaug[b][:, k, :],
                                         start=(k == 0), stop=(k == NKT - 1))
                    rs = smallp.tile([128, 1], f32, name="rs", tag="rs")
                    nc.vector.reciprocal(rs[:], up[:, HD:HD + 1])
                    un = smallp.tile([128, 128], bf16, name="un", tag="un",
                                     bufs=6)
                    nc.vector.tensor_scalar_mul(un[:], up[:, 0:HD], rs[:])
                    teng = nc.scalar if (tsplit and qtl % 2) else nc.sync
                    teng.dma_start_transpose(
                        ut[:, q0 + qtl * 128:q0 + (qtl + 1) * 128], un[:])

            # ---------- scope 1: projections woven with b0 attention ----------
            pk0 = {}   # (h, half) -> list of pk strips for b=0
            with (
                tc.tile_pool(name="wts", bufs=1) as wp,
                tc.tile_pool(name="xts", bufs=9) as xtp,
                tc.tile_pool(name="vt", bufs=1) as vtp,
                tc.tile_pool(name="pk0", bufs=18) as pk0p,
                tc.tile_pool(name="psA", bufs=2, space="PSUM") as psA,
            ):
                wq_sb = wp.tile([128, NT, HPC * HD], bf16, name="wq", tag="wq")
                wk_sb = wp.tile([128, NT, HD], bf16, name="wk", tag="wk")
                wv_sb = wp.tile([128, NT, HD], bf16, name="wv", tag="wv")
                vt = vtp.tile([128, S], bf16, name="vt", tag="vt", bufs=1)

                # PE warmup first: dummy matmuls keep the PE busy (and HAM
                # warming) from t~0 while the first DMAs land. Output
                # overwritten by the real O-projection later.
                wu = wp.tile([128, 512], bf16, name="wu", tag="wu")
                nc.vector.memset(wu[:], 0.0)
                pwu = psA.tile([128, 512], f32, name="pwu", tag="pp")
                for i in range(40):
                    nc.tensor.matmul(pwu[:, 0:128], wu[:, 0:128], wu[:, 0:128],
                                     start=(i == 0), stop=(i == 39))
                nc.vector.tensor_copy(wu[:, 0:128], pwu[:, 0:128])

                # ones columns for the fused softmax denominator
                for b in range(B):
                    nc.vector.memset(vaug[b][:, :, HD:HD + 1], 1.0)

                def load_chunk(b, c, head=False):
                    """6 group-DMAs of [128, 4 h-tiles, 512 cols] (512KB)."""
                    sl = slice(c * CW, (c + 1) * CW)
                    xts = []
                    for g in range(NG):
                        xtile = xtp.tile([128, GT, CW], bf16, name=f"x{g}",
                                         tag="x")
                        if head:
                            eng = [nc.scalar, nc.gpsimd, nc.sync][g % 3]
                        else:
                            eng = nc.gpsimd
                        eng.dma_start(xtile[:],
                                      xt_d[b, :, g * GT:(g + 1) * GT, sl])
                        xts.append(xtile)
                    return xts

                def load_half(b, hb):
                    """First-chunk halves: 3 DMAs of [128, 8 h-tiles, 256
                    cols] (512KB) so the first chain completes after only
                    ~1.9MB of fill instead of ~3.8MB."""
                    sl = slice(hb * 256, (hb + 1) * 256)
                    xts = []
                    for g in range(3):
                        xtile = xtp.tile([128, 8, 256], bf16, name=f"x0{g}",
                                         tag="x0", bufs=6)
                        eng = [nc.scalar, nc.gpsimd, nc.sync][g]
                        eng.dma_start(xtile[:],
                                      xt_d[b, :, g * 8:(g + 1) * 8, sl])
                        xts.append(xtile)
                    return xts

                GRPS = [HPC, HPC + 1, 0, 1, 2]  # K, V first, then q-heads

                def chain(b, xts, grp, c0, width, gdiv):
                    """One 24-matmul accumulation chain -> qt/kt/vt slice."""
                    pp = psA.tile([128, CW], f32, name="pp", tag="pp")
                    for t in range(NT):
                        if grp < HPC:
                            w_sl = wq_sb[:, t, grp * HD:(grp + 1) * HD]
                        elif grp == HPC:
                            w_sl = wk_sb[:, t, :]
                        else:
                            w_sl = wv_sb[:, t, :]
                        nc.tensor.matmul(pp[:, 0:width], w_sl,
                                         xts[t // gdiv][:, t % gdiv, :],
                                         start=(t == 0), stop=(t == NT - 1))
                    osl = slice(c0, c0 + width)
                    if grp < HPC:
                        nc.vector.tensor_copy(qt[b * HPC + grp][:, osl],
                                              pp[:, 0:width])
                    elif grp == HPC:
                        nc.vector.tensor_copy(kt[b][:, osl], pp[:, 0:width])
                    else:
                        nc.vector.tensor_copy(vt[:, osl], pp[:, 0:width])

                def v_fixup(b, c):
                    # XBAR transpose needs a 256B-aligned destination; stage
                    # at offset 0 and let gpsimd scatter into vaug.
                    for st in range(4 * c, 4 * c + 4):
                        tst = smallp.tile([128, 128], bf16, name="tst",
                                          tag="tst", bufs=4)
                        nc.sync.dma_start_transpose(
                            tst[:], vt[:, st * 128:(st + 1) * 128])
                        nc.gpsimd.tensor_copy(vaug[b][:, st, 0:HD], tst[:])

                # ---- DMA emission order: x first-chunk halves + wk first,
                # then wv/wq, then the rest of b0's x.
                nc.sync.dma_start(wk_sb[:], wk_d[:])
                x0a = load_half(0, 0)
                x0b = load_half(0, 1)
                nc.gpsimd.dma_start(wv_sb[:], wv_d[:])
                for wc in range(4):
                    eng = [nc.sync, nc.scalar, nc.gpsimd, nc.sync][wc]
                    eng.dma_start(wq_sb[:, wc * 6:(wc + 1) * 6, :],
                                  wq_d[:, wc * 6:(wc + 1) * 6, :])

                # b0 projection chunks (first chunk in two 256-col halves)
                for hb, xh in ((0, x0a), (1, x0b)):
                    for grp in GRPS:
                        chain(0, xh, grp, hb * 256, 256, 8)
                        if grp == HPC + 1 and hb == 1:
                            v_fixup(0, 0)
                for c in range(1, NCH):
                    xts = load_chunk(0, c, head=True)
                    for grp in GRPS:
                        chain(0, xts, grp, c * CW, CW, GT)
                        if grp == HPC + 1:
                            v_fixup(0, c)

                # b1 chains woven with b0 attention
                rest = []
                xts10 = load_chunk(1, 0)
                rest += [(1, 0, xts10, grp) for grp in GRPS]
                ri = [0]
                loaded = [1]  # b1 chunks loaded so far

                def emit_chains(n):
                    for _ in range(n):
                        if ri[0] < len(rest):
                            b, c, xts, grp = rest[ri[0]]
                            chain(b, xts, grp, c * CW, CW, GT)
                            if grp == HPC + 1:
                                v_fixup(1, c)
                            ri[0] += 1
                        # 2 chains into chunk c -> prefetch chunk c+1
                        if (loaded[0] < NCH
                                and ri[0] >= 5 * (loaded[0] - 1) + 2):
                            xts_n = load_chunk(1, loaded[0])
                            rest.extend((1, loaded[0], xts_n, grp)
                                        for grp in GRPS)
                            loaded[0] += 1

                # weave: b0 attention between b1 projection chains;
                # halves-first so O-proj chunks unblock in order.
                seq0 = [(h, hf) for hf in range(2) for h in range(HPC)]
                budget = [3, 3, 3, 3, 4, 4]
                for i, (h, half) in enumerate(seq0):
                    pk0[(h, half)] = [None] * NKT
                    scores_block(pk0p, pk0[(h, half)], 0, h, half)
                    emit_chains(budget[i])
                    pv_block(pk0[(h, half)], ut0[h], 0, half)
                    if i == 0:
                        # preload wo for scope 2's O-projection; gated on
                        # b1's first K-chain output so its 2.4MB cannot be
                        # hoisted into the startup's x fill
                        nc.vector.tensor_copy(wo_sb[:, 0, 0:1],
                                              kt[1][:, 0:1])
                        nc.gpsimd.dma_start(wo_sb[:], wo_d[:])
                assert ri[0] == 20 and len(rest) == 20 and loaded[0] == NCH

            # ---------- scope 2: b0 O-proj woven with b1 attention ----------
            with (
                tc.tile_pool(name="ut1", bufs=1) as utp1,
                tc.tile_pool(name="pk1", bufs=50) as pk1p,
                tc.tile_pool(name="ost", bufs=3) as ostp,
                tc.tile_pool(name="psO", bufs=2, space="PSUM") as psO,
            ):
                ut1 = [utp1.tile([128, S], bf16, name=f"u1{h}", tag="u1",
                                 bufs=HPC)
                       for h in range(HPC)]

                def oproj_chunk(ut, b, sc, late=False, split_tail=False):
                    """One 128-query row block x full H output.

                    One output DMA per row block (6KB/partition descriptors).
                    During b=0's O-proj the scalar engine is saturated with
                    b=1's exp stream, so those PSUM copies stay on vector.
                    Once the exps drain (`late` b=0 chunks, all b=1 chunks)
                    most copies move to scalar so the vector queue stays
                    clear for the pv normalize chains, and the by-then-idle
                    psS score banks give a 4-deep PSUM rotation.
                    split_tail: issue the store in two halves on separate
                    queues to shorten the end-of-kernel drain.
                    """
                    ssl = slice(sc * 128, (sc + 1) * 128)
                    ob = ostp.tile([128, H], bf16, name="ob", tag="ob")
                    for q in range(2 * HPC):
                        if (b == 1 or late) and q % 2 == 1:
                            op = psS.tile([128, 1024], f32, name="stp",
                                          tag="st")[:, 0:512]
                        else:
                            op = psO.tile([128, 512], f32, name="op", tag="o")
                        n0 = q * 512
                        for dq in range(HPC):
                            nc.tensor.matmul(op[:], ut[dq][:, ssl],
                                             wo_sb[:, dq, n0:n0 + 512],
                                             start=(dq == 0),
                                             stop=(dq == HPC - 1))
                        on_scalar = (q % 3 != 0) if late else (
                            b == 1 and q % 2 == 1)
                        if on_scalar:
                            nc.scalar.copy(ob[:, n0:n0 + 512], op[:])
                        else:
                            nc.vector.tensor_copy(ob[:, n0:n0 + 512], op[:])
                        if split_tail and q == 2:
                            nc.gpsimd.dma_start(out_d[b, ssl, 0:1536],
                                                ob[:, 0:1536])
                    if split_tail:
                        nc.sync.dma_start(out_d[b, ssl, 1536:H],
                                          ob[:, 1536:H])
                    else:
                        deng = nc.gpsimd if (b == 0 or sc % 2 == 0) else nc.sync
                        deng.dma_start(out_d[b, ssl, :], ob[:])

                sc0 = [0]

                def oproj0(n, late=False):
                    for _ in range(n):
                        if sc0[0] < S // 128:
                            oproj_chunk(ut0, 0, sc0[0], late=late)
                            sc0[0] += 1

                seq1 = [(h, hf) for hf in range(2) for h in range(HPC)]
                pk1 = {}

                def scores1(i):
                    pk1[seq1[i]] = [None] * NKT
                    scores_block(pk1p, pk1[seq1[i]], 1, *seq1[i])

                scores1(0)
                oproj0(2)
                scores1(1)
                oproj0(2)
                for i in range(2, 6):
                    scores1(i)
                    oproj0(2)
                    ph, phalf = seq1[i - 2]
                    pv_block(pk1[(ph, phalf)], ut1[ph], 1, phalf)
                oproj0(2, late=True)
                pv_block(pk1[seq1[4]], ut1[seq1[4][0]], 1, seq1[4][1],
                         tsplit=True)
                oproj0(2, late=True)
                pv_block(pk1[seq1[5]], ut1[seq1[5][0]], 1, seq1[5][1],
                         tsplit=True)
                oproj0(S // 128, late=True)  # remainder of b0, if any
                for sc in range(S // 128):
                    oproj_chunk(ut1, 1, sc, split_tail=(sc >= 12))

    nc.compile()
    return nc


def kernel(hidden_states, attention_mask, Wq, Wk, Wv, Wo):
    import os
    import tempfile

    from concourse.bass_utils import run_bass_kernel_spmd

    # the neuron compile hook drops a scratch file into cwd
    if not os.access(os.getcwd(), os.W_OK):
        os.chdir(tempfile.mkdtemp())

    if "nc" not in _CACHE:
        _CACHE["nc"] = _build()
    nc = _CACHE["nc"]

    in_maps = _prep_inputs(hidden_states, Wq, Wk, Wv, Wo)
    res = run_bass_kernel_spmd(nc, in_maps, core_ids=list(range(8)))
    out = np.zeros((B, S, H), dtype=np.float32)
    for r in res.results:
        out += r["out"].astype(np.float32)
    return out


def _prep_inputs(hidden_states, Wq, Wk, Wv, Wo):
    bf = ml_dtypes.bfloat16
    hs = np.asarray(hidden_states, dtype=np.float32)
    # xt[b, p, t, s] = hs[b, s, t*128 + p]
    xt = np.ascontiguousarray(
        hs.transpose(0, 2, 1).reshape(B, NT, 128, S).transpose(0, 2, 1, 3)
    ).astype(bf)
    Wq = np.asarray(Wq, dtype=np.float32)
    Wk = np.asarray(Wk, dtype=np.float32)
    Wv = np.asarray(Wv, dtype=np.float32)
    Wo = np.asarray(Wo, dtype=np.float32)

    def wslice(W, c, width):
        # [H, width] -> [128, NT, width] partition-major
        ws = W[:, c * width:(c + 1) * width]
        return np.ascontiguousarray(
            ws.reshape(NT, 128, width).transpose(1, 0, 2)).astype(bf)

    in_maps = []
    for c in range(8):
        wo = Wo[c * HPC * HD:(c + 1) * HPC * HD, :]  # [384, H]
        wo = np.ascontiguousarray(
            wo.reshape(HPC, 128, H).transpose(1, 0, 2)).astype(bf)
        in_maps.append({
            "xt": xt,
            "wq": wslice(Wq, c, HPC * HD),
            "wk": wslice(Wk, c, HD),
            "wv": wslice(Wv, c, HD),
            "wo": wo,
        })
    return in_maps


# revision 31
# speedup vs baseline: 1.0086x; 1.0077x over previous
"""GQA attention (Llama-style) on 8 Trainium2 NeuronCores.

Tensor-parallel over heads: core c owns q-heads [3c, 3c+1, 3c+2] and KV
head c. Each core computes a partial output contribution via its slice of
Wo (row-parallel); the host sums the 8 partials.

All matmul operands are bf16 (rel err ~6e-3 vs the fp32 reference, well
under the 2e-2 gate). Inputs are pre-laid-out on the host partition-major
so every DMA descriptor is >=1KB contiguous. Transposes (V and the
attention output) run on the DMA engines' XBAR path instead of the PE.

Schedule notes (v2):
- Projections are chunked 512 seq-cols at a time (8 chunks) so the first
  chain only needs 3MB of x + wk before it can finish; DMA emission is
  ordered x-first/wk-first across the three queues so the PE starts real
  work at ~3us instead of ~39us (and HAM stays warm).
- x tile pool holds 2 full chunks so prefetch never blocks on recycle.
- wo is preloaded during scope 1 so the b0 O-projection can fill the
  scope seam while b1's first score exps stream.
- Attention (h, half) units are emitted halves-first so O-proj chunks
  0-7 unblock after the first three pv_blocks.
- The last two output chunks split their store DMA in half across two
  queues to shorten the drain tail.

Shapes (hardcoded per the problem spec):
  hidden_states [2, 2048, 3072] f32, attention_mask [2,1,2048,2048] (zeros),
  Wq [3072, 3072], Wk/Wv [3072, 1024], Wo [3072, 3072] -> out [2, 2048, 3072].
"""

import ml_dtypes
import numpy as np

B, S, H = 2, 2048, 3072
NH, NKV, HD = 24, 8, 128
HPC = NH // 8        # q-heads per core
NT = H // 128        # 24 h-tiles of the hidden dim
NKT = S // 128       # 16 k-tiles of the sequence
NCH = 4              # projection chunks per batch
CW = S // NCH        # 512 seq-cols per chunk
NG = 6               # x DMA groups per chunk (512KB each)
GT = NT // NG        # 4 h-tiles per group
SCALE = float(1.0 / np.sqrt(HD))

_CACHE = {}


def _build():
    import concourse.mybir as mybir
    import concourse.tile as tile
    from concourse import bacc

    f32 = mybir.dt.float32
    bf16 = mybir.dt.bfloat16
    Exp = mybir.ActivationFunctionType.Exp

    nc = bacc.Bacc(None, target_bir_lowering=False)

    # Host pre-transposed, partition-major layouts (see _prep_inputs()).
    xt_d = nc.dram_tensor("xt", [B, 128, NT, S], bf16, kind="ExternalInput")
    wq_d = nc.dram_tensor("wq", [128, NT, HPC * HD], bf16, kind="ExternalInput")
    wk_d = nc.dram_tensor("wk", [128, NT, HD], bf16, kind="ExternalInput")
    wv_d = nc.dram_tensor("wv", [128, NT, HD], bf16, kind="ExternalInput")
    wo_d = nc.dram_tensor("wo", [128, HPC, H], bf16, kind="ExternalInput")
    out_d = nc.dram_tensor("out", [B, S, H], bf16, kind="ExternalOutput")

    with tile.TileContext(nc) as tc:
        with (
            tc.tile_pool(name="qkv", bufs=1) as qkvp,
            tc.tile_pool(name="ut0", bufs=1) as utp0,
            tc.tile_pool(name="wop", bufs=1) as wop,
            tc.tile_pool(name="small", bufs=4) as smallp,
            tc.tile_pool(name="psS", bufs=2, space="PSUM") as psS,
            tc.tile_pool(name="psU", bufs=2, space="PSUM") as psU,
        ):
            # Persistent per-(b,head) projections; partition dim is head_dim.
            qt = [qkvp.tile([128, S], bf16, name=f"qt{i}", tag="qt", bufs=B * HPC)
                  for i in range(B * HPC)]
            kt = [qkvp.tile([128, S], bf16, name=f"kt{i}", tag="kt", bufs=B)
                  for i in range(B)]
            # V with a fused ones column: [s-tile partition, k-tile, 129]
            vaug = [qkvp.tile([128, NKT, HD + 1], bf16, name=f"va{i}", tag="va",
                              bufs=B)
                    for i in range(B)]
            ut0 = [utp0.tile([128, S], bf16, name=f"u0{h}", tag="u0", bufs=HPC)
                   for h in range(HPC)]
            # wo lives in the outer scope so it can be preloaded in scope 1
            # and used by the O-projection in scope 2.
            wo_sb = wop.tile([128, HPC, H], bf16, name="wo", tag="wo")

            # ---------- emission helpers ----------
            def scores_block(pkp, pk_out, b, h, half):
                """Q@K^T for 1024 queries; exp on ACT -> pk strips (bf16)."""
                q0 = half * 1024
                qi = b * HPC + h
                for k in range(NKT):
                    stp = psS.tile([128, 1024], f32, name="stp", tag="st")
                    ksl = kt[b][:, k * 128:(k + 1) * 128]
                    nc.tensor.matmul(stp[:, 0:512], ksl,
                                     qt[qi][:, q0:q0 + 512],
                                     start=True, stop=True)
                    nc.tensor.matmul(stp[:, 512:1024], ksl,
                                     qt[qi][:, q0 + 512:q0 + 1024],
                                     start=True, stop=True)
                    pk = pkp.tile([128, 1024], bf16, name="pk", tag="pk")
                    nc.scalar.activation(pk[:], stp[:], Exp, scale=SCALE)
                    pk_out[k] = pk

            def pv_block(pks, ut, b, half, tsplit=False):
                """P@V_aug for 1024 queries; normalize; XBAR-transpose to ut.

                tsplit: alternate the transposes across sync+scalar (only
                safe once the scalar engine's exp stream has drained).
                """
                q0 = half * 1024
                for qtl in range(8):
                    up = psU.tile([128, HD + 1], f32, name="up", tag="u")
                    for k in range(NKT):
                        nc.tensor.matmul(up[:],
                                         pks[k][:, qtl * 128:(qtl + 1) * 128],
                                         vaug[b][:, k, :],
                                         start=(k == 0), stop=(k == NKT - 1))
                    rs = smallp.tile([128, 1], f32, name="rs", tag="rs")
                    nc.vector.reciprocal(rs[:], up[:, HD:HD + 1])
                    un = smallp.tile([128, 128], bf16, name="un", tag="un",
                                     bufs=6)
                    nc.vector.tensor_scalar_mul(un[:], up[:, 0:HD], rs[:])
                    teng = nc.scalar if (tsplit and qtl % 2) else nc.sync
                    teng.dma_start_transpose(
                        ut[:, q0 + qtl * 128:q0 + (qtl + 1) * 128], un[:])

            # ---------- scope 1: projections woven with b0 attention ----------
            pk0 = {}   # (h, half) -> list of pk strips for b=0
            with (
                tc.tile_pool(name="wts", bufs=1) as wp,
                tc.tile_pool(name="xts", bufs=2 * NG) as xtp,
                tc.tile_pool(name="vt", bufs=1) as vtp,
                tc.tile_pool(name="pk0", bufs=18) as pk0p,
                tc.tile_pool(name="psA", bufs=2, space="PSUM") as psA,
            ):
                wq_sb = wp.tile([128, NT, HPC * HD], bf16, name="wq", tag="wq")
                wk_sb = wp.tile([128, NT, HD], bf16, name="wk", tag="wk")
                wv_sb = wp.tile([128, NT, HD], bf16, name="wv", tag="wv")
                vt = vtp.tile([128, S], bf16, name="vt", tag="vt", bufs=1)

                # PE warmup first: dummy matmuls keep the PE busy (and HAM
                # warming) from t~0 while the first DMAs land. Output
                # overwritten by the real O-projection later.
                wu = wp.tile([128, 512], bf16, name="wu", tag="wu")
                nc.vector.memset(wu[:], 0.0)
                pwu = psA.tile([128, 512], f32, name="pwu", tag="pp")
                for i in range(32):
                    nc.tensor.matmul(pwu[:, 0:128], wu[:, 0:128], wu[:, 0:128],
                                     start=(i == 0), stop=(i == 31))
                nc.vector.tensor_copy(wu[:, 0:128], pwu[:, 0:128])
                nc.sync.dma_start(out_d[0, 0:128, 0:128], wu[:, 0:128])

                # ones columns for the fused softmax denominator
                for b in range(B):
                    nc.vector.memset(vaug[b][:, :, HD:HD + 1], 1.0)

                def load_chunk(b, c, head=False):
                    """6 group-DMAs of [128, 4 h-tiles, 512 cols] (512KB)."""
                    sl = slice(c * CW, (c + 1) * CW)
                    xts = []
                    for g in range(NG):
                        xtile = xtp.tile([128, GT, CW], bf16, name=f"x{g}",
                                         tag="x")
                        if head:
                            eng = [nc.scalar, nc.gpsimd, nc.sync][g % 3]
                        else:
                            eng = nc.gpsimd
                        eng.dma_start(xtile[:],
                                      xt_d[b, :, g * GT:(g + 1) * GT, sl])
                        xts.append(xtile)
                    return xts

                GRPS = [HPC, HPC + 1, 0, 1, 2]  # K, V first, then q-heads

                def chain(b, xts, grp, c0, width, gdiv):
                    """One 24-matmul accumulation chain -> qt/kt/vt slice."""
                    pp = psA.tile([128, CW], f32, name="pp", tag="pp")
                    for t in range(NT):
                        if grp < HPC:
                            w_sl = wq_sb[:, t, grp * HD:(grp + 1) * HD]
                        elif grp == HPC:
                            w_sl = wk_sb[:, t, :]
                        else:
                            w_sl = wv_sb[:, t, :]
                        nc.tensor.matmul(pp[:, 0:width], w_sl,
                                         xts[t // gdiv][:, t % gdiv, :],
                                         start=(t == 0), stop=(t == NT - 1))
                    osl = slice(c0, c0 + width)
                    if grp < HPC:
                        nc.vector.tensor_copy(qt[b * HPC + grp][:, osl],
                                              pp[:, 0:width])
                    elif grp == HPC:
                        nc.vector.tensor_copy(kt[b][:, osl], pp[:, 0:width])
                    else:
                        nc.vector.tensor_copy(vt[:, osl], pp[:, 0:width])

                def v_fixup(b, c):
                    # XBAR transpose needs a 256B-aligned destination; stage
                    # at offset 0 and let gpsimd scatter into vaug.
                    for st in range(4 * c, 4 * c + 4):
                        tst = smallp.tile([128, 128], bf16, name="tst",
                                          tag="tst", bufs=4)
                        nc.sync.dma_start_transpose(
                            tst[:], vt[:, st * 128:(st + 1) * 128])
                        nc.gpsimd.tensor_copy(vaug[b][:, st, 0:HD], tst[:])

                # ---- DMA emission order: x chunk 0 + wk first, then wv/wq
                nc.sync.dma_start(wk_sb[:], wk_d[:])
                xts00 = load_chunk(0, 0, head=True)
                nc.gpsimd.dma_start(wv_sb[:], wv_d[:])
                for wc in range(4):
                    eng = [nc.sync, nc.scalar, nc.gpsimd, nc.sync][wc]
                    eng.dma_start(wq_sb[:, wc * 6:(wc + 1) * 6, :],
                                  wq_d[:, wc * 6:(wc + 1) * 6, :])

                # b0 projection chunks
                for grp in GRPS:
                    chain(0, xts00, grp, 0, CW, GT)
                    if grp == HPC + 1:
                        v_fixup(0, 0)
                for c in range(1, NCH):
                    xts = load_chunk(0, c, head=True)
                    for grp in GRPS:
                        chain(0, xts, grp, c * CW, CW, GT)
                        if grp == HPC + 1:
                            v_fixup(0, c)

                # preload wo for scope 2's O-projection
                nc.gpsimd.dma_start(wo_sb[:], wo_d[:])

                # b1 chains woven with b0 attention
                rest = []
                xts10 = load_chunk(1, 0)
                rest += [(1, 0, xts10, grp) for grp in GRPS]
                ri = [0]
                loaded = [1]  # b1 chunks loaded so far

                def emit_chains(n):
                    for _ in range(n):
                        if ri[0] < len(rest):
                            b, c, xts, grp = rest[ri[0]]
                            chain(b, xts, grp, c * CW, CW, GT)
                            if grp == HPC + 1:
                                v_fixup(1, c)
                            ri[0] += 1
                        # 2 chains into chunk c -> prefetch chunk c+1
                        if (loaded[0] < NCH
                                and ri[0] >= 5 * (loaded[0] - 1) + 2):
                            xts_n = load_chunk(1, loaded[0])
                            rest.extend((1, loaded[0], xts_n, grp)
                                        for grp in GRPS)
                            loaded[0] += 1

                # weave: b0 attention between b1 projection chains;
                # halves-first so O-proj chunks unblock in order.
                seq0 = [(h, hf) for hf in range(2) for h in range(HPC)]
                budget = [3, 3, 3, 3, 4, 4]
                for i, (h, half) in enumerate(seq0):
                    pk0[(h, half)] = [None] * NKT
                    scores_block(pk0p, pk0[(h, half)], 0, h, half)
                    emit_chains(budget[i])
                    pv_block(pk0[(h, half)], ut0[h], 0, half)
                assert ri[0] == 20 and len(rest) == 20 and loaded[0] == NCH

            # ---------- scope 2: b0 O-proj woven with b1 attention ----------
            with (
                tc.tile_pool(name="ut1", bufs=1) as utp1,
                tc.tile_pool(name="pk1", bufs=50) as pk1p,
                tc.tile_pool(name="ost", bufs=3) as ostp,
                tc.tile_pool(name="psO", bufs=2, space="PSUM") as psO,
            ):
                ut1 = [utp1.tile([128, S], bf16, name=f"u1{h}", tag="u1",
                                 bufs=HPC)
                       for h in range(HPC)]

                def oproj_chunk(ut, b, sc, late=False, split_tail=False):
                    """One 128-query row block x full H output.

                    One output DMA per row block (6KB/partition descriptors).
                    During b=0's O-proj the scalar engine is saturated with
                    b=1's exp stream, so those PSUM copies stay on vector.
                    Once the exps drain (`late` b=0 chunks, all b=1 chunks)
                    most copies move to scalar so the vector queue stays
                    clear for the pv normalize chains, and the by-then-idle
                    psS score banks give a 4-deep PSUM rotation.
                    split_tail: issue the store in two halves on separate
                    queues to shorten the end-of-kernel drain.
                    """
                    ssl = slice(sc * 128, (sc + 1) * 128)
                    ob = ostp.tile([128, H], bf16, name="ob", tag="ob")
                    for q in range(2 * HPC):
                        if (b == 1 or late) and q % 2 == 1:
                            op = psS.tile([128, 1024], f32, name="stp",
                                          tag="st")[:, 0:512]
                        else:
                            op = psO.tile([128, 512], f32, name="op", tag="o")
                        n0 = q * 512
                        for dq in range(HPC):
                            nc.tensor.matmul(op[:], ut[dq][:, ssl],
                                             wo_sb[:, dq, n0:n0 + 512],
                                             start=(dq == 0),
                                             stop=(dq == HPC - 1))
                        on_scalar = (q % 3 != 0) if late else (
                            b == 1 and q % 2 == 1)
                        if on_scalar:
                            nc.scalar.copy(ob[:, n0:n0 + 512], op[:])
                        else:
                            nc.vector.tensor_copy(ob[:, n0:n0 + 512], op[:])
                        if split_tail and q == 2:
                            nc.gpsimd.dma_start(out_d[b, ssl, 0:1536],
                                                ob[:, 0:1536])
                    if split_tail:
                        nc.sync.dma_start(out_d[b, ssl, 1536:H],
                                          ob[:, 1536:H])
                    else:
                        deng = nc.gpsimd if (b == 0 or sc % 2 == 0) else nc.sync
                        deng.dma_start(out_d[b, ssl, :], ob[:])

                sc0 = [0]

                def oproj0(n, late=False):
                    for _ in range(n):
                        if sc0[0] < S // 128:
                            oproj_chunk(ut0, 0, sc0[0], late=late)
                            sc0[0] += 1

                seq1 = [(h, hf) for hf in range(2) for h in range(HPC)]
                pk1 = {}

                def scores1(i):
                    pk1[seq1[i]] = [None] * NKT
                    scores_block(pk1p, pk1[seq1[i]], 1, *seq1[i])

                scores1(0)
                oproj0(2)
                scores1(1)
                oproj0(2)
                for i in range(2, 6):
                    scores1(i)
                    oproj0(2)
                    ph, phalf = seq1[i - 2]
                    pv_block(pk1[(ph, phalf)], ut1[ph], 1, phalf)
                oproj0(2, late=True)
                pv_block(pk1[seq1[4]], ut1[seq1[4][0]], 1, seq1[4][1],
                         tsplit=True)
                oproj0(2, late=True)
                pv_block(pk1[seq1[5]], ut1[seq1[5][0]], 1, seq1[5][1],
                         tsplit=True)
                oproj0(S // 128, late=True)  # remainder of b0, if any
                for sc in range(S // 128):
                    oproj_chunk(ut1, 1, sc, split_tail=(sc >= 14))

    nc.compile()
    return nc


def kernel(hidden_states, attention_mask, Wq, Wk, Wv, Wo):
    import os
    import tempfile

    from concourse.bass_utils import run_bass_kernel_spmd

    # the neuron compile hook drops a scratch file into cwd
    if not os.access(os.getcwd(), os.W_OK):
        os.chdir(tempfile.mkdtemp())

    if "nc" not in _CACHE:
        _CACHE["nc"] = _build()
    nc = _CACHE["nc"]

    in_maps = _prep_inputs(hidden_states, Wq, Wk, Wv, Wo)
    res = run_bass_kernel_spmd(nc, in_maps, core_ids=list(range(8)))
    out = np.zeros((B, S, H), dtype=np.float32)
    for r in res.results:
        out += r["out"].astype(np.float32)
    return out


def _prep_inputs(hidden_states, Wq, Wk, Wv, Wo):
    bf = ml_dtypes.bfloat16
    hs = np.asarray(hidden_states, dtype=np.float32)
    # xt[b, p, t, s] = hs[b, s, t*128 + p]
    xt = np.ascontiguousarray(
        hs.transpose(0, 2, 1).reshape(B, NT, 128, S).transpose(0, 2, 1, 3)
    ).astype(bf)
    Wq = np.asarray(Wq, dtype=np.float32)
    Wk = np.asarray(Wk, dtype=np.float32)
    Wv = np.asarray(Wv, dtype=np.float32)
    Wo = np.asarray(Wo, dtype=np.float32)

    def wslice(W, c, width):
        # [H, width] -> [128, NT, width] partition-major
        ws = W[:, c * width:(c + 1) * width]
        return np.ascontiguousarray(
            ws.reshape(NT, 128, width).transpose(1, 0, 2)).astype(bf)

    in_maps = []
    for c in range(8):
        wo = Wo[c * HPC * HD:(c + 1) * HPC * HD, :]  # [384, H]
        wo = np.ascontiguousarray(
            wo.reshape(HPC, 128, H).transpose(1, 0, 2)).astype(bf)
        in_maps.append({
            "xt": xt,
            "wq": wslice(Wq, c, HPC * HD),
            "wk": wslice(Wk, c, HD),
            "wv": wslice(Wv, c, HD),
            "wo": wo,
        })
    return in_maps


# revision 33
# speedup vs baseline: 1.0099x; 1.0013x over previous
"""GQA attention (Llama-style) on 8 Trainium2 NeuronCores.

Tensor-parallel over heads: core c owns q-heads [3c, 3c+1, 3c+2] and KV
head c. Each core computes a partial output contribution via its slice of
Wo (row-parallel); the host sums the 8 partials.

All matmul operands are bf16 (rel err ~6e-3 vs the fp32 reference, well
under the 2e-2 gate). Inputs are pre-laid-out on the host partition-major
so every DMA descriptor is >=1KB contiguous. Transposes (V and the
attention output) run on the DMA engines' XBAR path instead of the PE.

Schedule notes (v2):
- Projections are chunked 512 seq-cols at a time (8 chunks) so the first
  chain only needs 3MB of x + wk before it can finish; DMA emission is
  ordered x-first/wk-first across the three queues so the PE starts real
  work at ~3us instead of ~39us (and HAM stays warm).
- x tile pool holds 2 full chunks so prefetch never blocks on recycle.
- wo is preloaded during scope 1 so the b0 O-projection can fill the
  scope seam while b1's first score exps stream.
- Attention (h, half) units are emitted halves-first so O-proj chunks
  0-7 unblock after the first three pv_blocks.
- The last two output chunks split their store DMA in half across two
  queues to shorten the drain tail.

Shapes (hardcoded per the problem spec):
  hidden_states [2, 2048, 3072] f32, attention_mask [2,1,2048,2048] (zeros),
  Wq [3072, 3072], Wk/Wv [3072, 1024], Wo [3072, 3072] -> out [2, 2048, 3072].
"""

import ml_dtypes
import numpy as np

B, S, H = 2, 2048, 3072
NH, NKV, HD = 24, 8, 128
HPC = NH // 8        # q-heads per core
NT = H // 128        # 24 h-tiles of the hidden dim
NKT = S // 128       # 16 k-tiles of the sequence
NCH = 4              # projection chunks per batch
CW = S // NCH        # 512 seq-cols per chunk
NG = 6               # x DMA groups per chunk (512KB each)
GT = NT // NG        # 4 h-tiles per group
SCALE = float(1.0 / np.sqrt(HD))

_CACHE = {}


def _build():
    import concourse.mybir as mybir
    import concourse.tile as tile
    from concourse import bacc

    f32 = mybir.dt.float32
    bf16 = mybir.dt.bfloat16
    Exp = mybir.ActivationFunctionType.Exp

    nc = bacc.Bacc(None, target_bir_lowering=False)

    # Host pre-transposed, partition-major layouts (see _prep_inputs()).
    xt_d = nc.dram_tensor("xt", [B, 128, NT, S], bf16, kind="ExternalInput")
    wq_d = nc.dram_tensor("wq", [128, NT, HPC * HD], bf16, kind="ExternalInput")
    wk_d = nc.dram_tensor("wk", [128, NT, HD], bf16, kind="ExternalInput")
    wv_d = nc.dram_tensor("wv", [128, NT, HD], bf16, kind="ExternalInput")
    wo_d = nc.dram_tensor("wo", [128, HPC, H], bf16, kind="ExternalInput")
    out_d = nc.dram_tensor("out", [B, S, H], bf16, kind="ExternalOutput")

    with tile.TileContext(nc) as tc:
        with (
            tc.tile_pool(name="qkv", bufs=1) as qkvp,
            tc.tile_pool(name="ut0", bufs=1) as utp0,
            tc.tile_pool(name="wop", bufs=1) as wop,
            tc.tile_pool(name="small", bufs=4) as smallp,
            tc.tile_pool(name="psS", bufs=2, space="PSUM") as psS,
            tc.tile_pool(name="psU", bufs=2, space="PSUM") as psU,
        ):
            # Persistent per-(b,head) projections; partition dim is head_dim.
            qt = [qkvp.tile([128, S], bf16, name=f"qt{i}", tag="qt", bufs=B * HPC)
                  for i in range(B * HPC)]
            kt = [qkvp.tile([128, S], bf16, name=f"kt{i}", tag="kt", bufs=B)
                  for i in range(B)]
            # V with a fused ones column: [s-tile partition, k-tile, 129]
            vaug = [qkvp.tile([128, NKT, HD + 1], bf16, name=f"va{i}", tag="va",
                              bufs=B)
                    for i in range(B)]
            ut0 = [utp0.tile([128, S], bf16, name=f"u0{h}", tag="u0", bufs=HPC)
                   for h in range(HPC)]
            # wo lives in the outer scope so it can be preloaded in scope 1
            # and used by the O-projection in scope 2.
            wo_sb = wop.tile([128, HPC, H], bf16, name="wo", tag="wo")

            # ---------- emission helpers ----------
            def scores_block(pkp, pk_out, b, h, half):
                """Q@K^T for 1024 queries; exp on ACT -> pk strips (bf16)."""
                q0 = half * 1024
                qi = b * HPC + h
                for k in range(NKT):
                    stp = psS.tile([128, 1024], f32, name="stp", tag="st")
                    ksl = kt[b][:, k * 128:(k + 1) * 128]
                    nc.tensor.matmul(stp[:, 0:512], ksl,
                                     qt[qi][:, q0:q0 + 512],
                                     start=True, stop=True)
                    nc.tensor.matmul(stp[:, 512:1024], ksl,
                                     qt[qi][:, q0 + 512:q0 + 1024],
                                     start=True, stop=True)
                    pk = pkp.tile([128, 1024], bf16, name="pk", tag="pk")
                    nc.scalar.activation(pk[:], stp[:], Exp, scale=SCALE)
                    pk_out[k] = pk

            def pv_block(pks, ut, b, half, tsplit=False):
                """P@V_aug for 1024 queries; normalize; XBAR-transpose to ut.

                tsplit: alternate the transposes across sync+scalar (only
                safe once the scalar engine's exp stream has drained).
                """
                q0 = half * 1024
                for qtl in range(8):
                    up = psU.tile([128, HD + 1], f32, name="up", tag="u")
                    for k in range(NKT):
                        nc.tensor.matmul(up[:],
                                         pks[k][:, qtl * 128:(qtl + 1) * 128],
                                         vaug[b][:, k, :],
                                         start=(k == 0), stop=(k == NKT - 1))
                    rs = smallp.tile([128, 1], f32, name="rs", tag="rs")
                    nc.vector.reciprocal(rs[:], up[:, HD:HD + 1])
                    un = smallp.tile([128, 128], bf16, name="un", tag="un",
                                     bufs=6)
                    nc.vector.tensor_scalar_mul(un[:], up[:, 0:HD], rs[:])
                    teng = nc.scalar if (tsplit and qtl % 2) else nc.sync
                    teng.dma_start_transpose(
                        ut[:, q0 + qtl * 128:q0 + (qtl + 1) * 128], un[:])

            # ---------- scope 1: projections woven with b0 attention ----------
            pk0 = {}   # (h, half) -> list of pk strips for b=0
            with (
                tc.tile_pool(name="wts", bufs=1) as wp,
                tc.tile_pool(name="xts", bufs=2 * NG) as xtp,
                tc.tile_pool(name="vt", bufs=1) as vtp,
                tc.tile_pool(name="pk0", bufs=18) as pk0p,
                tc.tile_pool(name="psA", bufs=2, space="PSUM") as psA,
            ):
                wq_sb = wp.tile([128, NT, HPC * HD], bf16, name="wq", tag="wq")
                wk_sb = wp.tile([128, NT, HD], bf16, name="wk", tag="wk")
                wv_sb = wp.tile([128, NT, HD], bf16, name="wv", tag="wv")
                vt = vtp.tile([128, S], bf16, name="vt", tag="vt", bufs=1)

                # PE warmup first: dummy matmuls keep the PE busy (and HAM
                # warming) from t~0 while the first DMAs land. Output
                # overwritten by the real O-projection later.
                wu = wp.tile([128, 512], bf16, name="wu", tag="wu")
                nc.vector.memset(wu[:], 0.0)
                pwu = psA.tile([128, 512], f32, name="pwu", tag="pp")
                for i in range(32):
                    nc.tensor.matmul(pwu[:, 0:128], wu[:, 0:128], wu[:, 0:128],
                                     start=(i == 0), stop=(i == 31))
                nc.vector.tensor_copy(wu[:, 0:128], pwu[:, 0:128])
                nc.sync.dma_start(out_d[0, 0:128, 0:128], wu[:, 0:128])

                # ones columns for the fused softmax denominator
                for b in range(B):
                    nc.vector.memset(vaug[b][:, :, HD:HD + 1], 1.0)

                def load_chunk(b, c, head=False):
                    """6 group-DMAs of [128, 4 h-tiles, 512 cols] (512KB)."""
                    sl = slice(c * CW, (c + 1) * CW)
                    xts = []
                    for g in range(NG):
                        xtile = xtp.tile([128, GT, CW], bf16, name=f"x{g}",
                                         tag="x")
                        if head:
                            eng = [nc.scalar, nc.gpsimd, nc.sync][g % 3]
                        else:
                            eng = nc.gpsimd
                        eng.dma_start(xtile[:],
                                      xt_d[b, :, g * GT:(g + 1) * GT, sl])
                        xts.append(xtile)
                    return xts

                GRPS = [HPC, HPC + 1, 0, 1, 2]  # K, V first, then q-heads

                def chain(b, xts, grp, c0, width, gdiv):
                    """One 24-matmul accumulation chain -> qt/kt/vt slice."""
                    pp = psA.tile([128, CW], f32, name="pp", tag="pp")
                    for t in range(NT):
                        if grp < HPC:
                            w_sl = wq_sb[:, t, grp * HD:(grp + 1) * HD]
                        elif grp == HPC:
                            w_sl = wk_sb[:, t, :]
                        else:
                            w_sl = wv_sb[:, t, :]
                        nc.tensor.matmul(pp[:, 0:width], w_sl,
                                         xts[t // gdiv][:, t % gdiv, :],
                                         start=(t == 0), stop=(t == NT - 1))
                    osl = slice(c0, c0 + width)
                    if grp < HPC:
                        nc.vector.tensor_copy(qt[b * HPC + grp][:, osl],
                                              pp[:, 0:width])
                    elif grp == HPC:
                        nc.vector.tensor_copy(kt[b][:, osl], pp[:, 0:width])
                    else:
                        nc.vector.tensor_copy(vt[:, osl], pp[:, 0:width])

                def v_fixup(b, c):
                    # XBAR transpose needs a 256B-aligned destination; stage
                    # at offset 0 and let gpsimd scatter into vaug.
                    for st in range(4 * c, 4 * c + 4):
                        tst = smallp.tile([128, 128], bf16, name="tst",
                                          tag="tst", bufs=4)
                        nc.sync.dma_start_transpose(
                            tst[:], vt[:, st * 128:(st + 1) * 128])
                        nc.gpsimd.tensor_copy(vaug[b][:, st, 0:HD], tst[:])

                # ---- DMA emission order: x chunk 0 + wk first, then wv/wq
                nc.sync.dma_start(wk_sb[:], wk_d[:])
                xts00 = load_chunk(0, 0, head=True)
                nc.gpsimd.dma_start(wv_sb[:], wv_d[:])
                for wc in range(4):
                    eng = [nc.sync, nc.scalar, nc.gpsimd, nc.sync][wc]
                    eng.dma_start(wq_sb[:, wc * 6:(wc + 1) * 6, :],
                                  wq_d[:, wc * 6:(wc + 1) * 6, :])

                # b0 projection chunks
                for grp in GRPS:
                    chain(0, xts00, grp, 0, CW, GT)
                    if grp == HPC + 1:
                        v_fixup(0, 0)
                for c in range(1, NCH):
                    xts = load_chunk(0, c, head=True)
                    for grp in GRPS:
                        chain(0, xts, grp, c * CW, CW, GT)
                        if grp == HPC + 1:
                            v_fixup(0, c)

                # preload wo for scope 2's O-projection
                nc.gpsimd.dma_start(wo_sb[:], wo_d[:])

                # b1 chains woven with b0 attention
                rest = []
                xts10 = load_chunk(1, 0)
                rest += [(1, 0, xts10, grp) for grp in GRPS]
                ri = [0]
                loaded = [1]  # b1 chunks loaded so far

                def emit_chains(n):
                    for _ in range(n):
                        if ri[0] < len(rest):
                            b, c, xts, grp = rest[ri[0]]
                            chain(b, xts, grp, c * CW, CW, GT)
                            if grp == HPC + 1:
                                v_fixup(1, c)
                            ri[0] += 1
                        # 2 chains into chunk c -> prefetch chunk c+1
                        if (loaded[0] < NCH
                                and ri[0] >= 5 * (loaded[0] - 1) + 2):
                            xts_n = load_chunk(1, loaded[0])
                            rest.extend((1, loaded[0], xts_n, grp)
                                        for grp in GRPS)
                            loaded[0] += 1

                # weave: b0 attention between b1 projection chains;
                # halves-first so O-proj chunks unblock in order.
                seq0 = [(h, hf) for hf in range(2) for h in range(HPC)]
                budget = [3, 3, 3, 3, 4, 4]
                for i, (h, half) in enumerate(seq0):
                    pk0[(h, half)] = [None] * NKT
                    scores_block(pk0p, pk0[(h, half)], 0, h, half)
                    emit_chains(budget[i])
                    pv_block(pk0[(h, half)], ut0[h], 0, half)
                assert ri[0] == 20 and len(rest) == 20 and loaded[0] == NCH

            # ---------- scope 2: b0 O-proj woven with b1 attention ----------
            with (
                tc.tile_pool(name="ut1", bufs=1) as utp1,
                tc.tile_pool(name="pk1", bufs=50) as pk1p,
                tc.tile_pool(name="ost", bufs=3) as ostp,
                tc.tile_pool(name="psO", bufs=2, space="PSUM") as psO,
            ):
                ut1 = [utp1.tile([128, S], bf16, name=f"u1{h}", tag="u1",
                                 bufs=HPC)
                       for h in range(HPC)]

                def oproj_chunk(ut, b, sc, late=False, split_tail=False):
                    """One 128-query row block x full H output.

                    One output DMA per row block (6KB/partition descriptors).
                    During b=0's O-proj the scalar engine is saturated with
                    b=1's exp stream, so those PSUM copies stay on vector.
                    Once the exps drain (`late` b=0 chunks, all b=1 chunks)
                    most copies move to scalar so the vector queue stays
                    clear for the pv normalize chains, and the by-then-idle
                    psS score banks give a 4-deep PSUM rotation.
                    split_tail: issue the store in two halves on separate
                    queues to shorten the end-of-kernel drain.
                    """
                    ssl = slice(sc * 128, (sc + 1) * 128)
                    tail = b == 1 and sc >= 10
                    ob = ostp.tile([128, H], bf16, name="ob", tag="ob")
                    for q in range(2 * HPC):
                        if (b == 1 or late) and q % 2 == 1:
                            op = psS.tile([128, 1024], f32, name="stp",
                                          tag="st")[:, 0:512]
                        else:
                            op = psO.tile([128, 512], f32, name="op", tag="o")
                        n0 = q * 512
                        for dq in range(HPC):
                            nc.tensor.matmul(op[:], ut[dq][:, ssl],
                                             wo_sb[:, dq, n0:n0 + 512],
                                             start=(dq == 0),
                                             stop=(dq == HPC - 1))
                        if tail:
                            # end of kernel: scalar carries a transpose/copy
                            # backlog while vector idles - shift load there
                            on_scalar = q in (1, 4)
                        elif late:
                            on_scalar = q % 3 != 0
                        else:
                            on_scalar = b == 1 and q % 2 == 1
                        if on_scalar:
                            nc.scalar.copy(ob[:, n0:n0 + 512], op[:])
                        else:
                            nc.vector.tensor_copy(ob[:, n0:n0 + 512], op[:])
                        if split_tail and q in (1, 3):
                            seg = slice((q // 2) * 1024, (q // 2 + 1) * 1024)
                            eng = nc.gpsimd if q == 1 else nc.sync
                            eng.dma_start(out_d[b, ssl, seg], ob[:, seg])
                    if split_tail:
                        nc.scalar.dma_start(out_d[b, ssl, 2048:H],
                                            ob[:, 2048:H])
                    else:
                        deng = nc.gpsimd if (b == 0 or sc % 2 == 0) else nc.sync
                        deng.dma_start(out_d[b, ssl, :], ob[:])

                sc0 = [0]

                def oproj0(n, late=False):
                    for _ in range(n):
                        if sc0[0] < S // 128:
                            oproj_chunk(ut0, 0, sc0[0], late=late)
                            sc0[0] += 1

                seq1 = [(h, hf) for hf in range(2) for h in range(HPC)]
                pk1 = {}

                def scores1(i):
                    pk1[seq1[i]] = [None] * NKT
                    scores_block(pk1p, pk1[seq1[i]], 1, *seq1[i])

                scores1(0)
                oproj0(2)
                scores1(1)
                oproj0(2)
                for i in range(2, 6):
                    scores1(i)
                    oproj0(2)
                    ph, phalf = seq1[i - 2]
                    pv_block(pk1[(ph, phalf)], ut1[ph], 1, phalf)
                oproj0(2, late=True)
                pv_block(pk1[seq1[4]], ut1[seq1[4][0]], 1, seq1[4][1],
                         tsplit=True)
                oproj0(2, late=True)
                pv_block(pk1[seq1[5]], ut1[seq1[5][0]], 1, seq1[5][1],
                         tsplit=True)
                oproj0(S // 128, late=True)  # remainder of b0, if any
                for sc in range(S // 128):
                    oproj_chunk(ut1, 1, sc, split_tail=(sc >= 14))

    nc.compile()
    return nc


def kernel(hidden_states, attention_mask, Wq, Wk, Wv, Wo):
    import os
    import tempfile

    from concourse.bass_utils import run_bass_kernel_spmd

    # the neuron compile hook drops a scratch file into cwd
    if not os.access(os.getcwd(), os.W_OK):
        os.chdir(tempfile.mkdtemp())

    if "nc" not in _CACHE:
        _CACHE["nc"] = _build()
    nc = _CACHE["nc"]

    in_maps = _prep_inputs(hidden_states, Wq, Wk, Wv, Wo)
    res = run_bass_kernel_spmd(nc, in_maps, core_ids=list(range(8)))
    out = np.zeros((B, S, H), dtype=np.float32)
    for r in res.results:
        out += r["out"].astype(np.float32)
    return out


def _prep_inputs(hidden_states, Wq, Wk, Wv, Wo):
    bf = ml_dtypes.bfloat16
    hs = np.asarray(hidden_states, dtype=np.float32)
    # xt[b, p, t, s] = hs[b, s, t*128 + p]
    xt = np.ascontiguousarray(
        hs.transpose(0, 2, 1).reshape(B, NT, 128, S).transpose(0, 2, 1, 3)
    ).astype(bf)
    Wq = np.asarray(Wq, dtype=np.float32)
    Wk = np.asarray(Wk, dtype=np.float32)
    Wv = np.asarray(Wv, dtype=np.float32)
    Wo = np.asarray(Wo, dtype=np.float32)

    def wslice(W, c, width):
        # [H, width] -> [128, NT, width] partition-major
        ws = W[:, c * width:(c + 1) * width]
        return np.ascontiguousarray(
            ws.reshape(NT, 128, width).transpose(1, 0, 2)).astype(bf)

    in_maps = []
    for c in range(8):
        wo = Wo[c * HPC * HD:(c + 1) * HPC * HD, :]  # [384, H]
        wo = np.ascontiguousarray(
            wo.reshape(HPC, 128, H).transpose(1, 0, 2)).astype(bf)
        in_maps.append({
            "xt": xt,
            "wq": wslice(Wq, c, HPC * HD),
            "wk": wslice(Wk, c, HD),
            "wv": wslice(Wv, c, HD),
            "wo": wo,
        })
    return in_maps


# revision 34
# speedup vs baseline: 1.0212x; 1.0112x over previous
"""GQA attention (Llama-style) on 8 Trainium2 NeuronCores.

Tensor-parallel over heads: core c owns q-heads [3c, 3c+1, 3c+2] and KV
head c. Each core computes a partial output contribution via its slice of
Wo (row-parallel); the host sums the 8 partials.

All matmul operands are bf16 (rel err ~6e-3 vs the fp32 reference, well
under the 2e-2 gate). Inputs are pre-laid-out on the host partition-major
so every DMA descriptor is >=1KB contiguous. Transposes (V and the
attention output) run on the DMA engines' XBAR path instead of the PE.

Schedule notes (v2):
- Projections are chunked 512 seq-cols at a time (8 chunks) so the first
  chain only needs 3MB of x + wk before it can finish; DMA emission is
  ordered x-first/wk-first across the three queues so the PE starts real
  work at ~3us instead of ~39us (and HAM stays warm).
- x tile pool holds 2 full chunks so prefetch never blocks on recycle.
- wo is preloaded during scope 1 so the b0 O-projection can fill the
  scope seam while b1's first score exps stream.
- Attention (h, half) units are emitted halves-first so O-proj chunks
  0-7 unblock after the first three pv_blocks.
- The last two output chunks split their store DMA in half across two
  queues to shorten the drain tail.

Shapes (hardcoded per the problem spec):
  hidden_states [2, 2048, 3072] f32, attention_mask [2,1,2048,2048] (zeros),
  Wq [3072, 3072], Wk/Wv [3072, 1024], Wo [3072, 3072] -> out [2, 2048, 3072].
"""

import ml_dtypes
import numpy as np

B, S, H = 2, 2048, 3072
NH, NKV, HD = 24, 8, 128
HPC = NH // 8        # q-heads per core
NT = H // 128        # 24 h-tiles of the hidden dim
NKT = S // 128       # 16 k-tiles of the sequence
NCH = 4              # projection chunks per batch
CW = S // NCH        # 512 seq-cols per chunk
NG = 6               # x DMA groups per chunk (512KB each)
GT = NT // NG        # 4 h-tiles per group
SCALE = float(1.0 / np.sqrt(HD))

_CACHE = {}


def _build():
    import concourse.mybir as mybir
    import concourse.tile as tile
    from concourse import bacc

    f32 = mybir.dt.float32
    bf16 = mybir.dt.bfloat16
    Exp = mybir.ActivationFunctionType.Exp

    nc = bacc.Bacc(None, target_bir_lowering=False)

    # Host pre-transposed, partition-major layouts (see _prep_inputs()).
    xt_d = nc.dram_tensor("xt", [B, 128, NT, S], bf16, kind="ExternalInput")
    wq_d = nc.dram_tensor("wq", [128, NT, HPC * HD], bf16, kind="ExternalInput")
    wk_d = nc.dram_tensor("wk", [128, NT, HD], bf16, kind="ExternalInput")
    wv_d = nc.dram_tensor("wv", [128, NT, HD], bf16, kind="ExternalInput")
    wo_d = nc.dram_tensor("wo", [128, HPC, H], bf16, kind="ExternalInput")
    out_d = nc.dram_tensor("out", [B, S, H], bf16, kind="ExternalOutput")

    with tile.TileContext(nc) as tc:
        with (
            tc.tile_pool(name="qkv", bufs=1) as qkvp,
            tc.tile_pool(name="ut0", bufs=1) as utp0,
            tc.tile_pool(name="wop", bufs=1) as wop,
            tc.tile_pool(name="small", bufs=4) as smallp,
            tc.tile_pool(name="psS", bufs=2, space="PSUM") as psS,
            tc.tile_pool(name="psU", bufs=2, space="PSUM") as psU,
        ):
            # Persistent per-(b,head) projections; partition dim is head_dim.
            qt = [qkvp.tile([128, S], bf16, name=f"qt{i}", tag="qt", bufs=B * HPC)
                  for i in range(B * HPC)]
            kt = [qkvp.tile([128, S], bf16, name=f"kt{i}", tag="kt", bufs=B)
                  for i in range(B)]
            # V with a fused ones column: [s-tile partition, k-tile, 129]
            vaug = [qkvp.tile([128, NKT, HD + 1], bf16, name=f"va{i}", tag="va",
                              bufs=B)
                    for i in range(B)]
            ut0 = [utp0.tile([128, S], bf16, name=f"u0{h}", tag="u0", bufs=HPC)
                   for h in range(HPC)]
            # wo lives in the outer scope so it can be preloaded in scope 1
            # and used by the O-projection in scope 2.
            wo_sb = wop.tile([128, HPC, H], bf16, name="wo", tag="wo")

            # ---------- emission helpers ----------
            def scores_block(pkp, pk_out, b, h, half):
                """Q@K^T for 1024 queries; exp on ACT -> pk strips (bf16)."""
                q0 = half * 1024
                qi = b * HPC + h
                for k in range(NKT):
                    stp = psS.tile([128, 1024], f32, name="stp", tag="st")
                    ksl = kt[b][:, k * 128:(k + 1) * 128]
                    nc.tensor.matmul(stp[:, 0:512], ksl,
                                     qt[qi][:, q0:q0 + 512],
                                     start=True, stop=True)
                    nc.tensor.matmul(stp[:, 512:1024], ksl,
                                     qt[qi][:, q0 + 512:q0 + 1024],
                                     start=True, stop=True)
                    pk = pkp.tile([128, 1024], bf16, name="pk", tag="pk")
                    nc.scalar.activation(pk[:], stp[:], Exp, scale=SCALE)
                    pk_out[k] = pk

            def pv_block(pks, ut, b, half, tsplit=False):
                """P@V_aug for 1024 queries; normalize; XBAR-transpose to ut.

                tsplit: alternate the transposes across sync+scalar (only
                safe once the scalar engine's exp stream has drained).
                """
                q0 = half * 1024
                for qtl in range(8):
                    up = psU.tile([128, HD + 1], f32, name="up", tag="u")
                    for k in range(NKT):
                        nc.tensor.matmul(up[:],
                                         pks[k][:, qtl * 128:(qtl + 1) * 128],
                                         vaug[b][:, k, :],
                                         start=(k == 0), stop=(k == NKT - 1))
                    rs = smallp.tile([128, 1], f32, name="rs", tag="rs")
                    nc.vector.reciprocal(rs[:], up[:, HD:HD + 1])
                    un = smallp.tile([128, 128], bf16, name="un", tag="un",
                                     bufs=6)
                    nc.vector.tensor_scalar_mul(un[:], up[:, 0:HD], rs[:])
                    teng = nc.scalar if (tsplit and qtl % 2) else nc.sync
                    teng.dma_start_transpose(
                        ut[:, q0 + qtl * 128:q0 + (qtl + 1) * 128], un[:])

            # ---------- scope 1: projections woven with b0 attention ----------
            pk0 = {}   # (h, half) -> list of pk strips for b=0
            with (
                tc.tile_pool(name="wts", bufs=1) as wp,
                tc.tile_pool(name="xts", bufs=2 * NG) as xtp,
                tc.tile_pool(name="vt", bufs=1) as vtp,
                tc.tile_pool(name="pk0", bufs=18) as pk0p,
                tc.tile_pool(name="psA", bufs=2, space="PSUM") as psA,
            ):
                wq_sb = wp.tile([128, NT, HPC * HD], bf16, name="wq", tag="wq")
                wk_sb = wp.tile([128, NT, HD], bf16, name="wk", tag="wk")
                wv_sb = wp.tile([128, NT, HD], bf16, name="wv", tag="wv")
                vt = vtp.tile([128, S], bf16, name="vt", tag="vt", bufs=1)

                # PE warmup first: dummy matmuls keep the PE busy (and HAM
                # warming) from t~0 while the first DMAs land. Output
                # overwritten by the real O-projection later.
                wu = wp.tile([128, 512], bf16, name="wu", tag="wu")
                nc.vector.memset(wu[:], 0.0)
                pwu = psA.tile([128, 512], f32, name="pwu", tag="pp")
                for i in range(32):
                    nc.tensor.matmul(pwu[:, 0:128], wu[:, 0:128], wu[:, 0:128],
                                     start=(i == 0), stop=(i == 31))
                nc.vector.tensor_copy(wu[:, 0:128], pwu[:, 0:128])
                nc.sync.dma_start(out_d[0, 0:128, 0:128], wu[:, 0:128])

                # ones columns for the fused softmax denominator
                for b in range(B):
                    nc.vector.memset(vaug[b][:, :, HD:HD + 1], 1.0)

                def load_chunk(b, c, head=False):
                    """6 group-DMAs of [128, 4 h-tiles, 512 cols] (512KB)."""
                    sl = slice(c * CW, (c + 1) * CW)
                    xts = []
                    for g in range(NG):
                        xtile = xtp.tile([128, GT, CW], bf16, name=f"x{g}",
                                         tag="x")
                        if head:
                            eng = [nc.scalar, nc.gpsimd, nc.sync][g % 3]
                        else:
                            eng = nc.gpsimd
                        eng.dma_start(xtile[:],
                                      xt_d[b, :, g * GT:(g + 1) * GT, sl])
                        xts.append(xtile)
                    return xts

                GRPS = [HPC, HPC + 1, 0, 1, 2]  # K, V first, then q-heads

                def chain(b, xts, grp, c0, width, gdiv):
                    """One 24-matmul accumulation chain -> qt/kt/vt slice."""
                    pp = psA.tile([128, CW], f32, name="pp", tag="pp")
                    for t in range(NT):
                        if grp < HPC:
                            w_sl = wq_sb[:, t, grp * HD:(grp + 1) * HD]
                        elif grp == HPC:
                            w_sl = wk_sb[:, t, :]
                        else:
                            w_sl = wv_sb[:, t, :]
                        nc.tensor.matmul(pp[:, 0:width], w_sl,
                                         xts[t // gdiv][:, t % gdiv, :],
                                         start=(t == 0), stop=(t == NT - 1))
                    osl = slice(c0, c0 + width)
                    if grp < HPC:
                        nc.vector.tensor_copy(qt[b * HPC + grp][:, osl],
                                              pp[:, 0:width])
                    elif grp == HPC:
                        nc.vector.tensor_copy(kt[b][:, osl], pp[:, 0:width])
                    else:
                        nc.vector.tensor_copy(vt[:, osl], pp[:, 0:width])

                def v_fixup(b, c):
                    # XBAR transpose needs a 256B-aligned destination; stage
                    # at offset 0 and let gpsimd scatter into vaug.
                    for st in range(4 * c, 4 * c + 4):
                        tst = smallp.tile([128, 128], bf16, name="tst",
                                          tag="tst", bufs=4)
                        nc.sync.dma_start_transpose(
                            tst[:], vt[:, st * 128:(st + 1) * 128])
                        nc.gpsimd.tensor_copy(vaug[b][:, st, 0:HD], tst[:])

                # ---- DMA emission order: x chunk 0 + wk first, then wv/wq
                nc.sync.dma_start(wk_sb[:], wk_d[:])
                xts00 = load_chunk(0, 0, head=True)
                nc.gpsimd.dma_start(wv_sb[:], wv_d[:])
                for wc in range(4):
                    eng = [nc.sync, nc.scalar, nc.gpsimd, nc.sync][wc]
                    eng.dma_start(wq_sb[:, wc * 6:(wc + 1) * 6, :],
                                  wq_d[:, wc * 6:(wc + 1) * 6, :])

                # b0 projection chunks
                for grp in GRPS:
                    chain(0, xts00, grp, 0, CW, GT)
                    if grp == HPC + 1:
                        v_fixup(0, 0)
                for c in range(1, NCH):
                    xts = load_chunk(0, c, head=True)
                    for grp in GRPS:
                        chain(0, xts, grp, c * CW, CW, GT)
                        if grp == HPC + 1:
                            v_fixup(0, c)

                # preload wo for scope 2's O-projection
                nc.gpsimd.dma_start(wo_sb[:], wo_d[:])

                # b1 chains woven with b0 attention
                rest = []
                xts10 = load_chunk(1, 0)
                rest += [(1, 0, xts10, grp) for grp in GRPS]
                ri = [0]
                loaded = [1]  # b1 chunks loaded so far

                def emit_chains(n):
                    for _ in range(n):
                        if ri[0] < len(rest):
                            b, c, xts, grp = rest[ri[0]]
                            chain(b, xts, grp, c * CW, CW, GT)
                            if grp == HPC + 1:
                                v_fixup(1, c)
                            ri[0] += 1
                        # 2 chains into chunk c -> prefetch chunk c+1
                        if (loaded[0] < NCH
                                and ri[0] >= 5 * (loaded[0] - 1) + 2):
                            xts_n = load_chunk(1, loaded[0])
                            rest.extend((1, loaded[0], xts_n, grp)
                                        for grp in GRPS)
                            loaded[0] += 1

                # weave: b0 attention between b1 projection chains;
                # halves-first so O-proj chunks unblock in order.
                seq0 = [(h, hf) for hf in range(2) for h in range(HPC)]
                budget = [3, 3, 3, 3, 4, 4]
                for i, (h, half) in enumerate(seq0):
                    pk0[(h, half)] = [None] * NKT
                    scores_block(pk0p, pk0[(h, half)], 0, h, half)
                    emit_chains(budget[i])
                    pv_block(pk0[(h, half)], ut0[h], 0, half)
                assert ri[0] == 20 and len(rest) == 20 and loaded[0] == NCH

            # ---------- scope 2: b0 O-proj woven with b1 attention ----------
            with (
                tc.tile_pool(name="ut1", bufs=1) as utp1,
                tc.tile_pool(name="pk1", bufs=50) as pk1p,
                tc.tile_pool(name="ost", bufs=3) as ostp,
                tc.tile_pool(name="psO", bufs=2, space="PSUM") as psO,
            ):
                ut1 = [utp1.tile([128, S], bf16, name=f"u1{h}", tag="u1",
                                 bufs=HPC)
                       for h in range(HPC)]

                def oproj_chunk(ut, b, sc, late=False, split_tail=False):
                    """One 128-query row block x full H output.

                    One output DMA per row block (6KB/partition descriptors).
                    During b=0's O-proj the scalar engine is saturated with
                    b=1's exp stream, so those PSUM copies stay on vector.
                    Once the exps drain (`late` b=0 chunks, all b=1 chunks)
                    most copies move to scalar so the vector queue stays
                    clear for the pv normalize chains, and the by-then-idle
                    psS score banks give a 4-deep PSUM rotation.
                    split_tail: issue the store in two halves on separate
                    queues to shorten the end-of-kernel drain.
                    """
                    ssl = slice(sc * 128, (sc + 1) * 128)
                    tail = b == 1 and sc >= 10
                    ob = ostp.tile([128, H], bf16, name="ob", tag="ob")
                    for q in range(2 * HPC):
                        if (b == 1 or late) and q % 2 == 1:
                            op = psS.tile([128, 1024], f32, name="stp",
                                          tag="st")[:, 0:512]
                        else:
                            op = psO.tile([128, 512], f32, name="op", tag="o")
                        n0 = q * 512
                        for dq in range(HPC):
                            nc.tensor.matmul(op[:], ut[dq][:, ssl],
                                             wo_sb[:, dq, n0:n0 + 512],
                                             start=(dq == 0),
                                             stop=(dq == HPC - 1))
                        if tail:
                            # end of kernel: scalar carries a transpose/copy
                            # backlog while vector idles - shift load there
                            on_scalar = q in (1, 4)
                        elif late:
                            on_scalar = q % 3 != 0
                        else:
                            on_scalar = b == 1 and q % 2 == 1
                        if on_scalar:
                            nc.scalar.copy(ob[:, n0:n0 + 512], op[:])
                        else:
                            nc.vector.tensor_copy(ob[:, n0:n0 + 512], op[:])
                        if split_tail and q in (1, 3):
                            seg = slice((q // 2) * 1024, (q // 2 + 1) * 1024)
                            eng = nc.gpsimd if q == 1 else nc.sync
                            eng.dma_start(out_d[b, ssl, seg], ob[:, seg])
                    if split_tail:
                        nc.scalar.dma_start(out_d[b, ssl, 2048:H],
                                            ob[:, 2048:H])
                    else:
                        deng = nc.gpsimd if (b == 0 or sc % 2 == 0) else nc.sync
                        deng.dma_start(out_d[b, ssl, :], ob[:])

                sc0 = [0]

                def oproj0(n, late=False):
                    for _ in range(n):
                        if sc0[0] < S // 128:
                            oproj_chunk(ut0, 0, sc0[0], late=late)
                            sc0[0] += 1

                seq1 = [(h, hf) for hf in range(2) for h in range(HPC)]
                pk1 = {}

                def scores1(i):
                    pk1[seq1[i]] = [None] * NKT
                    scores_block(pk1p, pk1[seq1[i]], 1, *seq1[i])

                scores1(0)
                oproj0(2)
                scores1(1)
                oproj0(2)
                for i in range(2, 6):
                    scores1(i)
                    oproj0(2)
                    ph, phalf = seq1[i - 2]
                    pv_block(pk1[(ph, phalf)], ut1[ph], 1, phalf)
                # b1 chunks 0-3 depend only on the half-0 pv_blocks (done
                # long ago) - interleave them here as ready PE work while
                # the last two pv_blocks wait out the ACT exp stream.
                oproj0(2, late=True)
                oproj_chunk(ut1, 1, 0)
                oproj_chunk(ut1, 1, 1)
                pv_block(pk1[seq1[4]], ut1[seq1[4][0]], 1, seq1[4][1],
                         tsplit=True)
                oproj0(2, late=True)
                oproj_chunk(ut1, 1, 2)
                oproj_chunk(ut1, 1, 3)
                pv_block(pk1[seq1[5]], ut1[seq1[5][0]], 1, seq1[5][1],
                         tsplit=True)
                oproj0(S // 128, late=True)  # remainder of b0, if any
                for sc in range(4, S // 128):
                    oproj_chunk(ut1, 1, sc, split_tail=(sc >= 14))

    nc.compile()
    return nc


def kernel(hidden_states, attention_mask, Wq, Wk, Wv, Wo):
    import os
    import tempfile

    from concourse.bass_utils import run_bass_kernel_spmd

    # the neuron compile hook drops a scratch file into cwd
    if not os.access(os.getcwd(), os.W_OK):
        os.chdir(tempfile.mkdtemp())

    if "nc" not in _CACHE:
        _CACHE["nc"] = _build()
    nc = _CACHE["nc"]

    in_maps = _prep_inputs(hidden_states, Wq, Wk, Wv, Wo)
    res = run_bass_kernel_spmd(nc, in_maps, core_ids=list(range(8)))
    out = np.zeros((B, S, H), dtype=np.float32)
    for r in res.results:
        out += r["out"].astype(np.float32)
    return out


def _prep_inputs(hidden_states, Wq, Wk, Wv, Wo):
    bf = ml_dtypes.bfloat16
    hs = np.asarray(hidden_states, dtype=np.float32)
    # xt[b, p, t, s] = hs[b, s, t*128 + p]
    xt = np.ascontiguousarray(
        hs.transpose(0, 2, 1).reshape(B, NT, 128, S).transpose(0, 2, 1, 3)
    ).astype(bf)
    Wq = np.asarray(Wq, dtype=np.float32)
    Wk = np.asarray(Wk, dtype=np.float32)
    Wv = np.asarray(Wv, dtype=np.float32)
    Wo = np.asarray(Wo, dtype=np.float32)

    def wslice(W, c, width):
        # [H, width] -> [128, NT, width] partition-major
        ws = W[:, c * width:(c + 1) * width]
        return np.ascontiguousarray(
            ws.reshape(NT, 128, width).transpose(1, 0, 2)).astype(bf)

    in_maps = []
    for c in range(8):
        wo = Wo[c * HPC * HD:(c + 1) * HPC * HD, :]  # [384, H]
        wo = np.ascontiguousarray(
            wo.reshape(HPC, 128, H).transpose(1, 0, 2)).astype(bf)
        in_maps.append({
            "xt": xt,
            "wq": wslice(Wq, c, HPC * HD),
            "wk": wslice(Wk, c, HD),
            "wv": wslice(Wv, c, HD),
            "wo": wo,
        })
    return in_maps
